# revision 1
# baseline (speedup 1.0000x reference)
"""ChebConv (R=4) Trainium2 kernel: 8-core batch-parallel, dense-streamed SpMM.

Sharding: batch dim B=16 -> 2 batches/core (F=256 features/core). Zero
collectives. Each core: 3 SpMM Chebyshev steps (y^T = x^T L^T on PE,
x stationary bf16, L^T streamed dense bf16), then einsum with W (contract
Cin) + bias, output (2, Cout, V) fp32 per core.
"""
import os
import sys

sys.path.insert(0, '/opt/trn_rl_repo')
sys.path.insert(0, '/opt/pypackages')

import numpy as np
import ml_dtypes

import concourse.bacc as bacc
import concourse.mybir as mybir
import concourse.tile as tile
from concourse import bass_utils

BF16 = mybir.dt.bfloat16
F32 = mybir.dt.float32

N_CORES = 8
SLAB = 8          # L windows per streaming DMA


def _shapes(V):
    NW = (V + 127) // 128          # src windows
    Vs = NW * 128
    NCH = (V + 511) // 512         # dest chunks
    Vd = NCH * 512
    return NW, Vs, NCH, Vd


def build_kernel(V, R=4, BL=2, CIN=128, COUT=128):
    """Build the per-core Bass module (SPMD: same NEFF on all cores)."""
    F = BL * CIN                   # 256
    FH = F // 128                  # 2 f-halves
    NW, Vs, NCH, Vd = _shapes(V)

    nc = bacc.Bacc("TRN2", target_bir_lowering=False, debug=False)

    x0vf = nc.dram_tensor("x0vf", [Vs, F], BF16, kind="ExternalInput")
    x0T = nc.dram_tensor("x0T", [F, Vd], BF16, kind="ExternalInput")
    lt = nc.dram_tensor("lt", [NCH, 128, NW, 512], BF16, kind="ExternalInput")
    wt = nc.dram_tensor("wt", [R, CIN, COUT], BF16, kind="ExternalInput")
    biasv = nc.dram_tensor("biasv", [COUT, 1], F32, kind="ExternalInput")
    yout = nc.dram_tensor("yout", [BL, COUT, V], F32, kind="ExternalOutput")

    xT = [nc.dram_tensor(f"xT{k}", [F, Vd], BF16) for k in (1, 2, 3)]

    with tile.TileContext(nc, trace_sim=False) as tc:
        with (
            tc.tile_pool(name="xp", bufs=1) as xp,
            tc.tile_pool(name="lp", bufs=4) as lp,
            tc.tile_pool(name="pp", bufs=8, space="PSUM") as pp,
            tc.tile_pool(name="cb", bufs=6) as cb,
            tc.tile_pool(name="pv", bufs=4) as pv,
            tc.tile_pool(name="ep", bufs=6) as ep,
            tc.tile_pool(name="eo", bufs=4) as eo,
            tc.tile_pool(name="wp", bufs=1) as wp,
        ):
            xcur = xp.tile([128, NW, F], BF16)
            nc.sync.dma_start(
                xcur[:], x0vf[:].rearrange("(w p) f -> p w f", p=128))

            wts = wp.tile([128, R, COUT], BF16)
            nc.sync.dma_start(wts[:], wt[:].rearrange("r i o -> i r o"))
            bias_sb = wp.tile([128, 1], F32)
            nc.sync.dma_start(bias_sb[:], biasv[:])

            n_slab = (NW + SLAB - 1) // SLAB

            for k in (1, 2, 3):
                prevT = None if k == 1 else (x0T if k == 2 else xT[0])
                for c in range(NCH):
                    ps = [pp.tile([128, 512], F32, tag="ps", name=f"ps_{k}_{c}_{h}") for h in range(FH)]
                    for s in range(n_slab):
                        w0 = s * SLAB
                        nw = min(SLAB, NW - w0)
                        ltile = lp.tile([128, SLAB, 512], BF16, tag="l")
                        nc.sync.dma_start(
                            ltile[:, :nw, :], lt[c, :, w0:w0 + nw, :])
                        for wl in range(nw):
                            w = w0 + wl
                            for h in range(FH):
                                nc.tensor.matmul(
                                    ps[h][:],
                                    lhsT=xcur[:, w, h * 128:(h + 1) * 128],
                                    rhs=ltile[:, wl, :],
                                    start=(w == 0),
                                    stop=(w == NW - 1),
                                )
                    for h in range(FH):
                        xo = cb.tile([128, 512], BF16, tag="cmb")
                        if k == 1:
                            nc.scalar.activation(
                                xo[:], ps[h][:],
                                mybir.ActivationFunctionType.Copy)
                        else:
                            tmp = cb.tile([128, 512], BF16, tag="cmb")
                            nc.vector.tensor_scalar_mul(tmp[:], ps[h][:], 2.0)
                            pvt = pv.tile([128, 512], BF16, tag="prev")
                            nc.sync.dma_start(
                                pvt[:],
                                prevT[h * 128:(h + 1) * 128,
                                      c * 512:(c + 1) * 512])
                            nc.vector.tensor_tensor(
                                out=xo[:], in0=tmp[:], in1=pvt[:],
                                op=mybir.AluOpType.subtract)
                        nc.sync.dma_start(
                            xT[k - 1][h * 128:(h + 1) * 128,
                                      c * 512:(c + 1) * 512], xo[:])
                if k < 3:
                    # reload xcur (v,f) from xT[k] via DMA transpose
                    for w in range(NW):
                        nc.sync.dma_start(
                            xcur[:, w, :],
                            xT[k - 1][:, w * 128:(w + 1) * 128],
                            transpose=True)

            # einsum: out[b, o, v] = sum_r W[r].T @ x_r^T  + bias
            xsrc = [x0T, xT[0], xT[1], xT[2]]
            for b in range(BL):
                for c in range(NCH):
                    eps = pp.tile([128, 512], F32, tag="ps")
                    for r in range(R):
                        xr = ep.tile([128, 512], BF16, tag="exr")
                        nc.sync.dma_start(
                            xr[:],
                            xsrc[r][b * 128:(b + 1) * 128,
                                    c * 512:(c + 1) * 512])
                        nc.tensor.matmul(
                            eps[:], lhsT=wts[:, r, :], rhs=xr[:],
                            start=(r == 0), stop=(r == R - 1))
                    ob = eo.tile([128, 512], F32, tag="eob")
                    nc.vector.tensor_scalar_add(ob[:], eps[:], bias_sb[:])
                    cols = min(512, V - c * 512)
                    nc.sync.dma_start(
                        yout[b, :, c * 512:c * 512 + cols], ob[:, :cols])

    nc.compile()
    return nc


def prep_inputs(x, weight, bias, lap_vals, lap_rows, lap_cols):
    """Host-side sharding + format conversion. Returns in_maps for 8 cores."""
    B, CIN, V = x.shape
    R = weight.shape[0]
    BL = B // N_CORES
    F = BL * CIN
    NW, Vs, NCH, Vd = _shapes(V)

    # dense L^T, padded: LT[src, dest] = L[dest, src]
    ltd = np.zeros((Vs, Vd), np.float32)
    np.add.at(ltd, (np.asarray(lap_cols), np.asarray(lap_rows)),
              np.asarray(lap_vals))
    lt_t = np.ascontiguousarray(
        ltd.reshape(NW, 128, NCH, 512).transpose(2, 1, 0, 3)
    ).astype(ml_dtypes.bfloat16)
    del ltd

    wt = np.asarray(weight, np.float32).astype(ml_dtypes.bfloat16)
    biasv = np.asarray(bias, np.float32).reshape(-1, 1)

    xf = np.asarray(x, np.float32)
    in_maps = []
    for c in range(N_CORES):
        xs = xf[c * BL:(c + 1) * BL]                  # (BL, CIN, V)
        x0 = np.transpose(xs, (2, 0, 1)).reshape(V, F)  # (V, F)
        x0p = np.zeros((Vs, F), np.float32)
        x0p[:V] = x0
        x0T = np.zeros((F, Vd), np.float32)
        x0T[:, :V] = x0.T
        in_maps.append({
            "x0vf": x0p.astype(ml_dtypes.bfloat16),
            "x0T": x0T.astype(ml_dtypes.bfloat16),
            "lt": lt_t,
            "wt": wt,
            "biasv": biasv,
        })
    return in_maps


_CACHE = {}


def get_built(V):
    if V not in _CACHE:
        _CACHE[V] = build_kernel(V)
    return _CACHE[V]


def kernel(x, weight, bias, lap_vals, lap_rows, lap_cols):
    B, CIN, V = x.shape
    BL = B // N_CORES
    nc = get_built(V)
    in_maps = prep_inputs(x, weight, bias, lap_vals, lap_rows, lap_cols)
    res = bass_utils.run_bass_kernel_spmd(
        nc, in_maps, core_ids=list(range(N_CORES)))
    out = np.concatenate([res.results[c]["yout"] for c in range(N_CORES)],
                         axis=0)
    return out.astype(np.float32)


if __name__ == "__main__":
    # smoke test vs numpy reference at reduced size
    V, NNZ, B, CIN, COUT, R = 1024, 2048, 16, 128, 128, 4
    rng = np.random.default_rng(0)
    x = rng.standard_normal((B, CIN, V)).astype(np.float32)
    weight = (rng.standard_normal((R, CIN, COUT)) *
              np.sqrt(2.0 / (R * CIN))).astype(np.float32)
    bias = np.full((COUT,), 0.01, np.float32)
    lap_vals = (rng.standard_normal(NNZ) / 32.0).astype(np.float32)
    lap_rows = rng.integers(0, V, NNZ).astype(np.int32)
    lap_cols = rng.integers(0, V, NNZ).astype(np.int32)

    def ref(x, weight, bias, lv, lr, lc):
        Vd_ = x.shape[2]
        L = np.zeros((Vd_, Vd_), np.float64)
        np.add.at(L, (lr, lc), lv.astype(np.float64))
        x0 = np.transpose(x, (2, 0, 1)).reshape(Vd_, -1).astype(np.float64)
        xs = [x0, L @ x0]
        for _ in range(R - 2):
            xs.append(2.0 * (L @ xs[-1]) - xs[-2])
        xs = np.stack(xs).reshape(R, Vd_, B, CIN)
        out = np.einsum('rvbi,rio->vbo', xs, weight.astype(np.float64))
        out = out + bias
        return np.transpose(out, (1, 2, 0)).astype(np.float32)

    expected = ref(x, weight, bias, lap_vals, lap_rows, lap_cols)
    got = kernel(x, weight, bias, lap_vals, lap_rows, lap_cols)
    err = np.abs(got - expected)
    scale = np.abs(expected).max()
    print("max abs err:", err.max(), "scale:", scale,
          "rel:", err.max() / scale)

